# revision 1
# baseline (speedup 1.0000x reference)
"""Trainium2 Bass kernel for nn_ConceptLayer (B=8, S=4096, D=64).

out[b,i,k] = LN( x[b,i,:] + sum_{a,c} x[b,i,a] * s_pre[b,i,c] * W[k,a,c] )
s_pre[b,i,c] = sum_{j<i} x[b,j,c] / (i-j)^2

Sharding: data-parallel over batch — one batch element per NeuronCore (8 cores).

Per-core algorithm (v4):
  One PSUM "megatile" (128, 4096) f32 spans all 8 banks; regions are carved
  manually (phases sequential per region, Tile tracks subtile deps).

  Phase A (PE): s2[c(+dup), 512-block] = sum_J x2[J].T @ TTS-slice (Toeplitz
    strip; causal diag) into megatile; DVE copy-cast -> s2b (bf16).
  Phase B, per (a,c)-chunk g (128 rows, a-major):
    xrep_g (host-replicated in DRAM) --plain DMA--> SBUF bf16
    outerT_g = xrep_g * s2b          (DVE 2x bf16; every 4th chunk on GPSIMD)
    outT[0:65, u] += W2TE_g.T @ outerT_g[:, u]  (PE; 65th output row carries
      sum_k out[i,k] via an extra all-ones-contracted weight column)
  Phase C: otb = outT + x.T (DVE); sq = otb^2 (DVE); sum_i(sq) via ones-col
    matmuls into a PSUM strip; strips -> SBUF (ACT) -> DMA-scatter to
    (128, 32) stat tiles; LN stats math on (128, 32) (DVE+ACT);
    per i-tile: PE-transpose otb -> (i, k), ACT applies (r-mu)*rstd via
    scale/bias, gamma/beta on GPSIMD, DMA out.
"""

import sys

sys.path.insert(0, "/opt/trn_rl_repo")

import numpy as np
import ml_dtypes

import concourse.bass as bass
import concourse.mybir as mybir
from concourse.tile import TileContext
from concourse.bass_utils import run_bass_kernel_spmd
from concourse.masks import make_identity

B, S, D = 8, 4096, 64
LN_EPS = 1e-3
P = 128
NT = S // P            # 32 i-tiles
NB = S // 512          # 8 512-blocks
NG = (D * D) // P      # 32 (a,c) chunks
NSTRIP = NB * 4 + 3    # 35 offset blocks in the Toeplitz strip

F32 = mybir.dt.float32
BF16 = mybir.dt.bfloat16
BF16_NP = ml_dtypes.bfloat16


# ---------------------------------------------------------------------------
# Workaround for walrus "Too many sync wait commands": this walrus build only
# accepts a single embedded sem wait per instruction. After Tile scheduling,
# split any instruction with N>1 waits into N-1 single-wait NOPs (same engine,
# placed just before it — identical blocking semantics).
def _split_multiwait(nc: bass.Bass, keep: int = 1):
    n = 0
    for fn in nc.m.functions:
        for bb in fn.blocks:
            insts = list(bb.instructions)
            out = []
            changed = False
            for inst in insts:
                si = inst.sync_info
                if si is not None and len(si.on_wait) > keep:
                    waits = list(si.on_wait)
                    for w in waits[:-keep]:
                        nop = mybir.InstNoOp(
                            name=f"WSPLIT-{n}", engine=inst.engine, ins=[], outs=[]
                        )
                        n += 1
                        nop.sync_info = mybir.SyncInfo(on_wait=[w], on_update=[])
                        out.append(nop)
                    inst.sync_info = mybir.SyncInfo(
                        on_wait=waits[-keep:], on_update=list(si.on_update)
                    )
                    changed = True
                out.append(inst)
            if changed:
                bb.instructions = out
    return n
# ---------------------------------------------------------------------------


def _host_constants(concept_map: np.ndarray):
    """Precompute host-side constant tensors (replicated across cores)."""
    # Toeplitz strip: TTS[q, 128*s + n] = f(128*(s-3) + n - q), f(v)=1/v^2 (v>0)
    q = np.arange(P)
    col = np.arange(NSTRIP * P)
    sblk, n_ = col // P, col % P
    v = 128 * (sblk[None, :] - 3) + n_[None, :] - q[:, None]
    tts = np.where(v > 0, 1.0 / np.maximum(v, 1).astype(np.float64) ** 2, 0.0)
    tts = tts.astype(np.float32)

    # W2TE[a*64+c, 0:64] = W[k, a, c]; [:, 64] = sum_k W[k, a, c]
    w2t = np.ascontiguousarray(
        concept_map.transpose(1, 2, 0).reshape(D * D, D)
    ).astype(np.float32)
    w2te = np.concatenate([w2t, w2t.sum(axis=1, keepdims=True)], axis=1)
    return tts.astype(BF16_NP), w2te.astype(BF16_NP)


def _build_nc(reps: int = 1, split: bool = True) -> bass.Bass:
    nc = bass.Bass("TRN2", target_bir_lowering=False, debug=False, num_devices=B)

    xb = nc.dram_tensor("xb", [S, D], F32, kind="ExternalInput")
    x2b = nc.dram_tensor("x2b", [S, 2 * D], BF16, kind="ExternalInput")
    xtb = nc.dram_tensor("xtb", [D, S], BF16, kind="ExternalInput")
    xrep_d = nc.dram_tensor("xrep", [NG, P, S], BF16, kind="ExternalInput")
    tts_d = nc.dram_tensor("tts", [P, NSTRIP * P], BF16, kind="ExternalInput")
    w2te_d = nc.dram_tensor("w2te", [D * D, D + 1], BF16, kind="ExternalInput")
    ones_d = nc.dram_tensor("ones64", [D, 1], BF16, kind="ExternalInput")
    xsum_d = nc.dram_tensor("xsum32", [P, NT], F32, kind="ExternalInput")
    gamma_d = nc.dram_tensor("gamma", [D], F32, kind="ExternalInput")
    beta_d = nc.dram_tensor("beta", [D], F32, kind="ExternalInput")
    y_d = nc.dram_tensor("y", [S, D], F32, kind="ExternalOutput")
    strip_d = nc.dram_tensor("strip_scratch", [2, S], F32)

    dma_engs = [nc.sync, nc.scalar]

    with TileContext(nc) as tc:
        with (
            tc.tile_pool(name="singles", bufs=1) as singles,
            tc.tile_pool(name="xrep", bufs=8) as xrep_pool,
            tc.tile_pool(name="outp", bufs=4) as out_pool,
            tc.tile_pool(name="eplg", bufs=8) as eplg,
            tc.tile_pool(name="psum", bufs=1, space="PSUM") as psum,
        ):

            def body():
                # ---- resident SBUF tiles ---------------------------------
                xf = singles.tile([P, NT, D], F32, tag="xf")
                nc.sync.dma_start(out=xf, in_=xb.rearrange("(j p) c -> p j c", p=P))
                x2t = singles.tile([P, NT, 2 * D], BF16, tag="x2t")
                nc.sync.dma_start(
                    out=x2t, in_=x2b.rearrange("(j p) c -> p j c", p=P)
                )
                xT = singles.tile([D, S], BF16, tag="xT")
                nc.sync.dma_start(out=xT, in_=xtb[:])
                tts = singles.tile([P, NSTRIP * P], BF16, tag="tts")
                nc.scalar.dma_start(out=tts, in_=tts_d[:])
                w2te = singles.tile([P, NG, D + 1], BF16, tag="w2te")
                nc.scalar.dma_start(
                    out=w2te, in_=w2te_d.rearrange("(g p) k -> p g k", p=P)
                )
                onescol = singles.tile([D, 1], BF16, tag="onescol")
                nc.scalar.dma_start(out=onescol, in_=ones_d[:])
                xsum32 = singles.tile([P, NT], F32, tag="xsum32")
                nc.scalar.dma_start(out=xsum32, in_=xsum_d[:])
                gam = singles.tile([P, D], F32, tag="gam")
                nc.scalar.dma_start(
                    out=gam,
                    in_=bass.AP(
                        tensor=gamma_d.ap().tensor,
                        offset=gamma_d.ap().offset,
                        ap=[[0, P], [1, D]],
                    ),
                )
                bet = singles.tile([P, D], F32, tag="bet")
                nc.scalar.dma_start(
                    out=bet,
                    in_=bass.AP(
                        tensor=beta_d.ap().tensor,
                        offset=beta_d.ap().offset,
                        ap=[[0, P], [1, D]],
                    ),
                )
                eps_t = singles.tile([P, 1], F32, tag="eps")
                nc.vector.memset(eps_t, LN_EPS)
                ident = singles.tile([P, P], F32, tag="ident")
                make_identity(nc, ident)

                s2b = singles.tile([P, S], BF16, tag="s2b")
                otb = singles.tile([D, S], F32, tag="otb")
                sqb = singles.tile([D, S], BF16, tag="sqb")
                strip0 = singles.tile([1, S], F32, tag="strip0")
                strip1 = singles.tile([1, S], F32, tag="strip1")

                mega = psum.tile([P, S], F32, tag="mega")

                # ---- Phase A: s_pre (PE) into megatile -------------------
                for ib in range(NB):
                    asl = slice(512 * ib, 512 * (ib + 1))
                    for J in range(4 * ib + 4):
                        s0 = 4 * ib - J + 3
                        nc.tensor.matmul(
                            mega[:, asl],
                            lhsT=x2t[:, J, :],
                            rhs=tts[:, 128 * s0 : 128 * s0 + 512],
                            start=(J == 0),
                            stop=(J == 4 * ib + 3),
                        )
                    nc.vector.tensor_copy(out=s2b[:, asl], in_=mega[:, asl])

                # ---- Phase B: product + bilinear into outT gang ----------
                for g in range(NG):
                    xr = xrep_pool.tile([P, S], BF16, tag="xrep")
                    dma_engs[g % 2].dma_start(out=xr, in_=xrep_d[g])
                    ot = out_pool.tile([P, S], BF16, tag="outerT")
                    if g % 4 == 3:
                        nc.gpsimd.tensor_mul(ot, xr, s2b)
                    else:
                        nc.vector.tensor_mul(ot, xr, s2b)
                    for u in range(NB):
                        nc.tensor.matmul(
                            mega[0 : D + 1, 512 * u : 512 * (u + 1)],
                            lhsT=w2te[:, g, :],
                            rhs=ot[:, 512 * u : 512 * (u + 1)],
                            start=(g == 0),
                            stop=(g == NG - 1),
                        )

                # ---- Phase C ---------------------------------------------
                # otb = outT + xT ; sq = otb^2 (bf16)
                for u in range(NB):
                    csl = slice(512 * u, 512 * (u + 1))
                    nc.vector.tensor_add(
                        otb[:, csl], mega[0:D, csl], xT[:, csl]
                    )
                    nc.vector.tensor_mul(sqb[:, csl], otb[:, csl], otb[:, csl])
                    # copy sum_k out strip (gang row 64) to SBUF first (ACT)
                    nc.scalar.copy(out=strip0[:, csl], in_=mega[D : D + 1, csl])
                    # sum_k r^2 strip reuses row 64 after the copy (WAR via Tile)
                    nc.tensor.matmul(
                        mega[D : D + 1, csl],
                        lhsT=onescol,
                        rhs=sqb[:, csl],
                        start=True,
                        stop=True,
                    )
                    nc.scalar.copy(out=strip1[:, csl], in_=mega[D : D + 1, csl])

                # scatter strips (1, 4096) -> (128, 32): dst[p, t] = strip[128t + p]
                # (bounce through DRAM so the transpose-ish AP balances)
                nc.sync.dma_start(out=strip_d[0:1, :], in_=strip0)
                nc.sync.dma_start(out=strip_d[1:2, :], in_=strip1)
                sumo32 = singles.tile([P, NT], F32, tag="sumo32")
                sumsq32 = singles.tile([P, NT], F32, tag="sumsq32")
                for k, dst in ((0, sumo32), (1, sumsq32)):
                    src = strip_d[k : k + 1, :]
                    src_b = bass.AP(
                        tensor=src.tensor,
                        offset=src.offset,
                        ap=[[1, P], [P, NT]],
                    )
                    nc.sync.dma_start(out=dst, in_=src_b)

                # LN stats on (128, 32): mu, rstd, -mu*rstd
                mu = singles.tile([P, NT], F32, tag="mu")
                nc.vector.tensor_add(mu, sumo32, xsum32)
                nc.vector.tensor_scalar_mul(out=mu, in0=mu, scalar1=1.0 / D)
                musq = singles.tile([P, NT], F32, tag="musq")
                nc.vector.tensor_mul(musq, mu, mu)
                var = singles.tile([P, NT], F32, tag="var")
                nc.vector.tensor_scalar_mul(out=var, in0=sumsq32, scalar1=1.0 / D)
                nc.vector.tensor_sub(var, var, musq)
                rstd = singles.tile([P, NT], F32, tag="rstd")
                nc.scalar.activation(
                    out=rstd,
                    in_=var,
                    func=mybir.ActivationFunctionType.Sqrt,
                    bias=eps_t,
                    scale=1.0,
                )
                nc.vector.reciprocal(out=rstd, in_=rstd)
                negmr = singles.tile([P, NT], F32, tag="negmr")
                nc.vector.tensor_mul(negmr, mu, rstd)
                nc.vector.tensor_scalar_mul(out=negmr, in0=negmr, scalar1=-1.0)

                # per-tile: transpose, normalize (ACT), gamma/beta (GPSIMD)
                for t in range(NT):
                    bk = t % NB
                    tsl = slice(512 * bk, 512 * bk + D)
                    nc.tensor.transpose(
                        mega[:, tsl],
                        in_=otb[:, 128 * t : 128 * (t + 1)],
                        identity=ident[0:D, 0:D],
                    )
                    y = eplg.tile([P, D], F32, tag="y")
                    nc.scalar.activation(
                        out=y,
                        in_=mega[:, tsl],
                        func=mybir.ActivationFunctionType.Identity,
                        bias=negmr[:, t : t + 1],
                        scale=rstd[:, t : t + 1],
                    )
                    nc.gpsimd.tensor_mul(y, y, gam)
                    nc.gpsimd.tensor_add(y, y, bet)
                    nc.sync.dma_start(out=y_d[128 * t : 128 * (t + 1), :], in_=y)

            if reps == 1:
                body()
            else:
                with tc.For_i(0, reps, 1):
                    body()

    if split:
        _split_multiwait(nc)
    return nc


def _make_in_maps(x, w, gamma, beta):
    tts, w2te = _host_constants(w)
    ones64 = np.ones((D, 1), BF16_NP)
    in_maps = []
    for b in range(B):
        xb = np.ascontiguousarray(x[b])
        xt = np.ascontiguousarray(xb.T).astype(BF16_NP)
        # xrep[g, p, :] = xT[2g + p//64, :]
        xrep = np.ascontiguousarray(
            xt.reshape(NG, 2, 1, S).repeat(D, axis=2).reshape(NG, P, S)
        )
        xsum = xb.sum(axis=1).astype(np.float32)  # (S,)
        xsum32 = np.ascontiguousarray(xsum.reshape(NT, P).T)  # [p, t] = xsum[128t+p]
        in_maps.append(
            {
                "xb": xb,
                "x2b": np.concatenate([xb, xb], axis=1).astype(BF16_NP),
                "xtb": xt,
                "xrep": xrep,
                "tts": tts,
                "w2te": w2te,
                "ones64": ones64,
                "xsum32": xsum32,
                "gamma": gamma,
                "beta": beta,
            }
        )
    return in_maps


_CACHED = {}


def kernel(**inputs: np.ndarray) -> np.ndarray:
    x = np.asarray(inputs["x"], np.float32)
    w = np.asarray(inputs["concept_map"], np.float32)
    gamma = np.asarray(inputs["gamma"], np.float32)
    beta = np.asarray(inputs["beta"], np.float32)
    assert x.shape == (B, S, D)

    if "nc" not in _CACHED:
        _CACHED["nc"] = _build_nc()
    nc = _CACHED["nc"]
    in_maps = _make_in_maps(x, w, gamma, beta)
    res = run_bass_kernel_spmd(nc, in_maps, core_ids=list(range(B)))
    return np.stack([res.results[b]["y"] for b in range(B)], axis=0)


if __name__ == "__main__":
    rng = np.random.default_rng(0)
    ins = {
        "x": rng.standard_normal((B, S, D), dtype=np.float32),
        "concept_map": (rng.standard_normal((D, D, D)) * 0.02).astype(np.float32),
        "gamma": np.ones(D, np.float32),
        "beta": np.zeros(D, np.float32),
    }
    y = kernel(**ins)
    print("ran", y.shape, y.dtype)



# revision 4
# speedup vs baseline: 1.2237x; 1.2237x over previous
"""Trainium2 Bass kernel for nn_ConceptLayer (B=8, S=4096, D=64).

out[b,i,k] = LN( x[b,i,:] + sum_{a,c} x[b,i,a] * s_pre[b,i,c] * W[k,a,c] )
s_pre[b,i,c] = sum_{j<i} x[b,j,c] / (i-j)^2

Sharding: data-parallel over batch - one batch element per NeuronCore (8 cores).

Per-core algorithm (v5.1):
  One PSUM "megatile" (128, 4096) f32 spans all 8 banks; regions are carved
  manually (phases sequential per region, Tile tracks subtile deps).

  Phase A (PE, BANDED): s2[c(+dup), 512-block] = sum_J x2[J].T @ TTS-slice
    (Toeplitz strip, causal diag). Band: only J-tiles within ~256-768 of the
    diagonal (1/d^2 tail beyond adds < 2e-4 rel err vs the 2e-2 gate).
    46 matmuls vs 144 dense. ACT copy-casts -> s2b (bf16).
  Phase B, per (a,c)-chunk g (128 rows, a-major):
    xrep_g (host-replicated in DRAM) --DMA (sync/scalar alternate)--> SBUF
    outerT_g = xrep_g * s2b          (DVE 2x bf16; every 3rd chunk on Pool)
    outT[0:65, u] += W2TE_g.T @ outerT_g[:, u]  (PE; 65th output row carries
      sum_k out[i,k] via an extra all-ones-contracted weight column)
  Phase C: otb = outT + x.T (DVE); sq = otb^2 (DVE); sum_i(sq) via ones-col
    matmuls into a PSUM strip; both strips collected in one (2,S) tile ->
    single DRAM bounce -> one scatter DMA to (128, 2, 32) stats; LN stats
    math (DVE+ACT); per i-tile: PE-transpose otb -> (i, k), normalize via
    per-partition affine (even tiles ACT, odd tiles DVE) into a (128,NT,D)
    y-buffer; optional gamma/beta on Pool (skipped when gamma==1, beta==0,
    which the problem spec guarantees); ONE contiguous output DMA, host
    unscrambles (p, t, k) -> (128*t+p, k).
"""

import sys

sys.path.insert(0, "/opt/trn_rl_repo")

import numpy as np
import ml_dtypes

import concourse.bass as bass
import concourse.mybir as mybir
from concourse.tile import TileContext
from concourse.bass_utils import run_bass_kernel_spmd
from concourse.masks import make_identity

B, S, D = 8, 4096, 64
LN_EPS = 1e-3
P = 128
NT = S // P            # 32 i-tiles
NB = S // 512          # 8 512-blocks
NG = (D * D) // P      # 32 (a,c) chunks
NSTRIP = 9             # banded Toeplitz strip: s0 in [0,5], +4 blocks window

F32 = mybir.dt.float32
BF16 = mybir.dt.bfloat16
BF16_NP = ml_dtypes.bfloat16


# ---------------------------------------------------------------------------
# Workaround for walrus "Too many sync wait commands": this walrus build only
# accepts a single embedded sem wait per instruction. After Tile scheduling,
# split any instruction with N>1 waits into N-1 single-wait NOPs (same engine,
# placed just before it - identical blocking semantics).
def _split_multiwait(nc: bass.Bass, keep: int = 1):
    n = 0
    for fn in nc.m.functions:
        for bb in fn.blocks:
            insts = list(bb.instructions)
            out = []
            changed = False
            for inst in insts:
                si = inst.sync_info
                if si is not None and len(si.on_wait) > keep:
                    waits = list(si.on_wait)
                    for w in waits[:-keep]:
                        nop = mybir.InstNoOp(
                            name=f"WSPLIT-{n}", engine=inst.engine, ins=[], outs=[]
                        )
                        n += 1
                        nop.sync_info = mybir.SyncInfo(on_wait=[w], on_update=[])
                        out.append(nop)
                    inst.sync_info = mybir.SyncInfo(
                        on_wait=waits[-keep:], on_update=list(si.on_update)
                    )
                    changed = True
                out.append(inst)
            if changed:
                bb.instructions = out
    return n
# ---------------------------------------------------------------------------


def _host_constants(concept_map: np.ndarray):
    """Precompute host-side constant tensors (replicated across cores)."""
    # Banded Toeplitz strip: TTS[q, 128*s + n] = f(128*(s-3) + n - q),
    # f(v) = 1/v^2 for v > 0 else 0.  s in [0, NSTRIP)
    q = np.arange(P)
    col = np.arange(NSTRIP * P)
    sblk, n_ = col // P, col % P
    v = 128 * (sblk[None, :] - 3) + n_[None, :] - q[:, None]
    tts = np.where(v > 0, 1.0 / np.maximum(v, 1).astype(np.float64) ** 2, 0.0)
    tts = tts.astype(np.float32)

    # W2TE[a*64+c, 0:64] = W[k, a, c]; [:, 64] = sum_k W[k, a, c]
    # prepacked p-major: (128, NG, 65) with [p, g, k] = W2TE[g*128+p, k]
    w2t = np.ascontiguousarray(
        concept_map.transpose(1, 2, 0).reshape(D * D, D)
    ).astype(np.float32)
    w2te = np.concatenate([w2t, w2t.sum(axis=1, keepdims=True)], axis=1)
    w2tp = np.ascontiguousarray(
        w2te.reshape(NG, P, D + 1).transpose(1, 0, 2)
    )
    return tts.astype(BF16_NP), w2tp.astype(BF16_NP)


def _build_nc(reps: int = 1, split: bool = True, apply_gb: bool = False) -> bass.Bass:
    nc = bass.Bass("TRN2", target_bir_lowering=False, debug=False, num_devices=B)

    # x2p: prepacked (128, NT*128) bf16, [p, j*128+c] = x[128j+p, c%64]
    x2p = nc.dram_tensor("x2p", [P, NT * 2 * D], BF16, kind="ExternalInput")
    xtb = nc.dram_tensor("xtb", [D, S], BF16, kind="ExternalInput")
    xrep_d = nc.dram_tensor("xrep", [NG, P, S], BF16, kind="ExternalInput")
    tts_d = nc.dram_tensor("tts", [P, NSTRIP * P], BF16, kind="ExternalInput")
    w2tp_d = nc.dram_tensor("w2tp", [P, NG * (D + 1)], BF16, kind="ExternalInput")
    ones_d = nc.dram_tensor("ones64", [D, 1], BF16, kind="ExternalInput")
    xsum_d = nc.dram_tensor("xsum32", [P, NT], F32, kind="ExternalInput")
    gamma_d = nc.dram_tensor("gamma", [D], F32, kind="ExternalInput")
    beta_d = nc.dram_tensor("beta", [D], F32, kind="ExternalInput")
    # output in (p, t, k) layout; host unscrambles to (128*t+p, k)
    y_d = nc.dram_tensor("yP", [P, NT * D], F32, kind="ExternalOutput")
    strip_d = nc.dram_tensor("strip_scratch", [2, S], F32)

    dma_engs = [nc.sync, nc.scalar]

    with TileContext(nc) as tc:
        with (
            tc.tile_pool(name="singles", bufs=1) as singles,
            tc.tile_pool(name="xrep", bufs=8) as xrep_pool,
            tc.tile_pool(name="outp", bufs=4) as out_pool,
            tc.tile_pool(name="psum", bufs=1, space="PSUM") as psum,
        ):

            def body():
                # ---- resident SBUF tiles ---------------------------------
                x2t = singles.tile([P, NT, 2 * D], BF16, tag="x2t")
                nc.sync.dma_start(out=x2t, in_=x2p[:])
                xT = singles.tile([D, S], BF16, tag="xT")
                nc.sync.dma_start(out=xT, in_=xtb[:])
                tts = singles.tile([P, NSTRIP * P], BF16, tag="tts")
                nc.scalar.dma_start(out=tts, in_=tts_d[:])
                w2te = singles.tile([P, NG, D + 1], BF16, tag="w2te")
                nc.scalar.dma_start(out=w2te, in_=w2tp_d[:])
                onescol = singles.tile([D, 1], BF16, tag="onescol")
                nc.scalar.dma_start(out=onescol, in_=ones_d[:])
                xsum32 = singles.tile([P, NT], F32, tag="xsum32")
                nc.scalar.dma_start(out=xsum32, in_=xsum_d[:])
                gam = singles.tile([P, D], F32, tag="gam")
                nc.scalar.dma_start(
                    out=gam,
                    in_=bass.AP(
                        tensor=gamma_d.ap().tensor,
                        offset=gamma_d.ap().offset,
                        ap=[[0, P], [1, D]],
                    ),
                )
                bet = singles.tile([P, D], F32, tag="bet")
                nc.scalar.dma_start(
                    out=bet,
                    in_=bass.AP(
                        tensor=beta_d.ap().tensor,
                        offset=beta_d.ap().offset,
                        ap=[[0, P], [1, D]],
                    ),
                )
                eps_t = singles.tile([P, 1], F32, tag="eps")
                nc.vector.memset(eps_t, LN_EPS)
                ident = singles.tile([P, P], F32, tag="ident")
                make_identity(nc, ident)

                s2b = singles.tile([P, S], BF16, tag="s2b")
                otb = singles.tile([D, S], F32, tag="otb")
                sqb = singles.tile([D, S], BF16, tag="sqb")
                strip0 = singles.tile([1, S], F32, tag="strip0")
                strip1 = singles.tile([1, S], F32, tag="strip1")
                ybuf = singles.tile([P, NT, D], F32, tag="ybuf")

                mega = psum.tile([P, S], F32, tag="mega")

                # ---- Phase A: banded s_pre (PE) into megatile ------------
                for ib in range(NB):
                    asl = slice(512 * ib, 512 * (ib + 1))
                    jlo = max(0, 4 * ib - 2)
                    for J in range(jlo, 4 * ib + 4):
                        s0 = 4 * ib - J + 3
                        nc.tensor.matmul(
                            mega[:, asl],
                            lhsT=x2t[:, J, :],
                            rhs=tts[:, 128 * s0 : 128 * s0 + 512],
                            start=(J == jlo),
                            stop=(J == 4 * ib + 3),
                        )
                    nc.scalar.copy(out=s2b[:, asl], in_=mega[:, asl])

                # ---- Phase B: product + bilinear into outT gang ----------
                for g in range(NG):
                    xr = xrep_pool.tile([P, S], BF16, tag="xrep")
                    dma_engs[g % 2].dma_start(out=xr, in_=xrep_d[g])
                    ot = out_pool.tile([P, S], BF16, tag="outerT")
                    if g % 3 == 2:
                        nc.gpsimd.tensor_mul(ot, xr, s2b)
                    else:
                        nc.vector.tensor_mul(ot, xr, s2b)
                    for u in range(NB):
                        nc.tensor.matmul(
                            mega[0 : D + 1, 512 * u : 512 * (u + 1)],
                            lhsT=w2te[:, g, :],
                            rhs=ot[:, 512 * u : 512 * (u + 1)],
                            start=(g == 0),
                            stop=(g == NG - 1),
                        )

                # ---- Phase C ---------------------------------------------
                # otb = outT + xT ; sq = otb^2 (bf16)
                for u in range(NB):
                    csl = slice(512 * u, 512 * (u + 1))
                    nc.vector.tensor_add(
                        otb[:, csl], mega[0:D, csl], xT[:, csl]
                    )
                    nc.vector.tensor_mul(sqb[:, csl], otb[:, csl], otb[:, csl])
                    # copy sum_k out strip (gang row 64) to SBUF first (ACT)
                    nc.scalar.copy(out=strip0[:, csl], in_=mega[D : D + 1, csl])
                    # sum_k r^2 strip reuses row 64 after the copy (WAR via Tile)
                    nc.tensor.matmul(
                        mega[D : D + 1, csl],
                        lhsT=onescol,
                        rhs=sqb[:, csl],
                        start=True,
                        stop=True,
                    )
                    nc.scalar.copy(out=strip1[:, csl], in_=mega[D : D + 1, csl])

                # one bounce: two rows -> DRAM -> one scatter (128, 2, NT)
                nc.sync.dma_start(out=strip_d[0:1, :], in_=strip0)
                nc.scalar.dma_start(out=strip_d[1:2, :], in_=strip1)
                stat2 = singles.tile([P, 2, NT], F32, tag="stat2")
                src = strip_d[:]
                src_b = bass.AP(
                    tensor=src.tensor,
                    offset=src.offset,
                    ap=[[1, P], [S, 2], [P, NT]],
                )
                nc.sync.dma_start(out=stat2, in_=src_b)
                sumo32 = stat2[:, 0, :]
                sumsq32 = stat2[:, 1, :]

                # LN stats on (128, 32): mu, rstd, -mu*rstd
                mu = singles.tile([P, NT], F32, tag="mu")
                nc.vector.tensor_add(mu, sumo32, xsum32)
                nc.vector.tensor_scalar_mul(out=mu, in0=mu, scalar1=1.0 / D)
                musq = singles.tile([P, NT], F32, tag="musq")
                nc.vector.tensor_mul(musq, mu, mu)
                var = singles.tile([P, NT], F32, tag="var")
                nc.vector.tensor_scalar_mul(out=var, in0=sumsq32, scalar1=1.0 / D)
                nc.vector.tensor_sub(var, var, musq)
                rstd = singles.tile([P, NT], F32, tag="rstd")
                nc.scalar.activation(
                    out=rstd,
                    in_=var,
                    func=mybir.ActivationFunctionType.Sqrt,
                    bias=eps_t,
                    scale=1.0,
                )
                nc.vector.reciprocal(out=rstd, in_=rstd)
                negmr = singles.tile([P, NT], F32, tag="negmr")
                nc.vector.tensor_mul(negmr, mu, rstd)
                nc.vector.tensor_scalar_mul(out=negmr, in0=negmr, scalar1=-1.0)

                # per-tile: transpose (PE), normalize (ACT/DVE alternate)
                for t in range(NT):
                    bk = t % NB
                    tsl = slice(512 * bk, 512 * bk + D)
                    nc.tensor.transpose(
                        mega[:, tsl],
                        in_=otb[:, 128 * t : 128 * (t + 1)],
                        identity=ident[0:D, 0:D],
                    )
                    ysl = ybuf[:, t, :]
                    if t % 2 == 0:
                        nc.scalar.activation(
                            out=ysl,
                            in_=mega[:, tsl],
                            func=mybir.ActivationFunctionType.Identity,
                            bias=negmr[:, t : t + 1],
                            scale=rstd[:, t : t + 1],
                        )
                    else:
                        nc.vector.tensor_scalar(
                            out=ysl,
                            in0=mega[:, tsl],
                            scalar1=rstd[:, t : t + 1],
                            scalar2=negmr[:, t : t + 1],
                            op0=mybir.AluOpType.mult,
                            op1=mybir.AluOpType.add,
                        )
                    if apply_gb:
                        nc.gpsimd.tensor_mul(ysl, ysl, gam)
                        nc.gpsimd.tensor_add(ysl, ysl, bet)
                nc.sync.dma_start(out=y_d[:], in_=ybuf)

            if reps == 1:
                body()
            else:
                with tc.For_i(0, reps, 1):
                    body()

    if split:
        _split_multiwait(nc)
    return nc


def _make_in_maps(x, w, gamma, beta):
    tts, w2tp = _host_constants(w)
    ones64 = np.ones((D, 1), BF16_NP)
    in_maps = []
    for b in range(B):
        xb = np.ascontiguousarray(x[b])
        xt = np.ascontiguousarray(xb.T).astype(BF16_NP)
        # xrep[g, p, :] = xT[2g + p//64, :]
        xrep = np.ascontiguousarray(
            xt.reshape(NG, 2, 1, S).repeat(D, axis=2).reshape(NG, P, S)
        )
        x2 = np.concatenate([xb, xb], axis=1).astype(BF16_NP)  # (S, 128)
        x2p = np.ascontiguousarray(
            x2.reshape(NT, P, 2 * D).transpose(1, 0, 2).reshape(P, NT * 2 * D)
        )
        xsum = xb.sum(axis=1).astype(np.float32)  # (S,)
        xsum32 = np.ascontiguousarray(xsum.reshape(NT, P).T)  # [p, t] = xsum[128t+p]
        in_maps.append(
            {
                "x2p": x2p,
                "xtb": xt,
                "xrep": xrep,
                "tts": tts,
                "w2tp": w2tp.reshape(P, NG * (D + 1)),
                "ones64": ones64,
                "xsum32": xsum32,
                "gamma": gamma,
                "beta": beta,
            }
        )
    return in_maps


_CACHED = {}


def kernel(**inputs: np.ndarray) -> np.ndarray:
    x = np.asarray(inputs["x"], np.float32)
    w = np.asarray(inputs["concept_map"], np.float32)
    gamma = np.asarray(inputs["gamma"], np.float32)
    beta = np.asarray(inputs["beta"], np.float32)
    assert x.shape == (B, S, D)

    apply_gb = not (
        np.allclose(gamma, 1.0, atol=0, rtol=0)
        and np.allclose(beta, 0.0, atol=0, rtol=0)
    )
    key = ("nc", apply_gb)
    if key not in _CACHED:
        _CACHED[key] = _build_nc(apply_gb=apply_gb)
    nc = _CACHED[key]
    in_maps = _make_in_maps(x, w, gamma, beta)
    res = run_bass_kernel_spmd(nc, in_maps, core_ids=list(range(B)))
    out = np.empty((B, S, D), np.float32)
    for b in range(B):
        yp = res.results[b]["yP"].reshape(P, NT, D)
        out[b] = yp.transpose(1, 0, 2).reshape(S, D)
    return out


if __name__ == "__main__":
    rng = np.random.default_rng(0)
    ins = {
        "x": rng.standard_normal((B, S, D), dtype=np.float32),
        "concept_map": (rng.standard_normal((D, D, D)) * 0.02).astype(np.float32),
        "gamma": np.ones(D, np.float32),
        "beta": np.zeros(D, np.float32),
    }
    y = kernel(**ins)
    print("ran", y.shape, y.dtype)
